# revision 14
# baseline (speedup 1.0000x reference)
"""Trainium2 Bass kernel for nn_JointSelfAttentionLayer.

Math restructuring (both outputs are sequence-means):
  C    = (1/(SC*32)) * (w @ x_d) @ W_vd,   w[t] = sum_s softmax(logits)[s,t]
  Dout = (1/(SD*32)) * (sum_s x_c) @ W_vc  (softmax rows sum to 1)
so the heavy work is logits = (x_c@W_qc) @ (x_d@W_kd)^T plus a streaming
softmax column-sum. Never materializes Q, K, V_c, V_d, or A@V.

All big matmuls run single-pass f16 (1 cyc/row on the PE, fp32 PSUM
accumulation): end-to-end rel err ~1.6e-3 vs the 2e-2 gate. Inputs load
fp32 over the two HWDGE queues, cast f16 on DVE, transpose on the PE.
"""
import numpy as np
from contextlib import ExitStack

B, SC, SD, D = 8, 2048, 2048, 1024
P = 128
DB = D // P            # 8 d-blocks
CH = 512
NCH = SC // CH         # 4 s-chunks
SBK = SC // P          # 16 s-blocks
TB = SD // P           # 16 t-blocks


def _split_excess_waits(nc, mybir, max_waits=1):
    n = 0
    ctr = [0]
    for fn in nc.m.functions:
        for bb in fn.blocks:
            out = []
            changed = False
            for inst in bb.instructions:
                si = inst.sync_info
                ws = list(si.on_wait) if (si and si.on_wait) else []
                if len(ws) > max_waits and inst.engine != mybir.EngineType.Unassigned:
                    keep = ws[:max_waits]
                    excess = ws[max_waits:]
                    for i in range(0, len(excess), max_waits):
                        chunk = excess[i:i + max_waits]
                        nop = mybir.InstNoOp(name=f"ws_{ctr[0]}", ins=[], outs=[])
                        ctr[0] += 1
                        nop.engine = inst.engine
                        nop.sync_info = mybir.SyncInfo(on_wait=chunk, on_update=[])
                        out.append(nop)
                    inst.sync_info = mybir.SyncInfo(
                        on_wait=keep, on_update=list(si.on_update or []))
                    changed = True
                    n += 1
                out.append(inst)
            if changed:
                bb.instructions = out
    return n


def _build():
    import concourse.bass as bass
    import concourse.tile as tile
    from concourse import mybir
    from concourse.masks import make_identity

    F32 = mybir.dt.float32
    F16 = mybir.dt.float16
    Act = mybir.ActivationFunctionType
    Alu = mybir.AluOpType
    AxX = mybir.AxisListType.X

    nc = bass.Bass("TRN2", target_bir_lowering=False, debug=False, num_devices=8)
    xc = nc.dram_tensor("x_c", [SC, D], F32, kind="ExternalInput").ap()
    xd = nc.dram_tensor("x_d", [SD, D], F32, kind="ExternalInput").ap()
    wqc = nc.dram_tensor("W_qc", [D, D], F32, kind="ExternalInput").ap()
    wvc = nc.dram_tensor("W_vc", [D, D], F32, kind="ExternalInput").ap()
    wkd = nc.dram_tensor("W_kd", [D, D], F32, kind="ExternalInput").ap()
    wvd = nc.dram_tensor("W_vd", [D, D], F32, kind="ExternalInput").ap()
    out_d = nc.dram_tensor("out", [1, 2 * D], F32, kind="ExternalOutput").ap()

    with tile.TileContext(nc) as tc, ExitStack() as ctx:
        const = ctx.enter_context(tc.tile_pool(name="const", bufs=1))
        ident = const.tile([P, P], F16, name="ident")
        make_identity(nc, ident[:])
        cp = const.tile([P, SD], F16, name="cp")
        nc.gpsimd.memset(cp[:], 0.0)
        wT16 = const.tile([P, TB], F16, name="wT16")
        uT16 = const.tile([P, DB], F16, name="uT16")
        u32 = const.tile([1, D], F32, name="u32")
        ones32 = const.tile([1, 1], F32, name="ones32")
        nc.gpsimd.memset(ones32[:], 1.0)
        dummy = const.tile([1, 1], F32, name="dummy")
        nc.scalar.activation(dummy[:], ones32[:], Act.Exp)
        out_sb = const.tile([1, 2 * D], F32, name="out_sb")

        qt_pool = ctx.enter_context(tc.tile_pool(name="qt", bufs=1))
        QT = [qt_pool.tile([P, SC], F16, name=f"QT{e}") for e in range(DB)]
        kt_pool = ctx.enter_context(tc.tile_pool(name="kt", bufs=1))
        KT = [kt_pool.tile([P, SD], F16, name=f"KT{e}") for e in range(DB)]
        xdn_pool = ctx.enter_context(tc.tile_pool(name="xdn", bufs=1))
        xdn = [xdn_pool.tile([P, D], F16, name=f"xdn{t}") for t in range(TB)]

        xcT_scope = tc.tile_pool(name="xcT", bufs=1)
        xcT_pool = xcT_scope.__enter__()

        # ---- phase Q: QT = W_qc^T @ xc^T ----
        with tc.tile_pool(name="xin", bufs=3) as xin, \
             tc.tile_pool(name="win", bufs=2) as winp, \
             tc.tile_pool(name="xdin", bufs=2) as xdinp, \
             tc.tile_pool(name="x16t", bufs=8) as x16t, \
             tc.tile_pool(name="wq16", bufs=1) as wq_pool, \
             tc.tile_pool(name="tpsq", bufs=3, space="PSUM") as tpsq, \
             tc.tile_pool(name="qps", bufs=4, space="PSUM") as qps:
            wq16 = [wq_pool.tile([P, D], F16, name=f"wq16_{f}") for f in range(DB)]
            xcT = [xcT_pool.tile([P, SC], F16, name=f"xcT{f}") for f in range(DB)]

            def load_cast_xc(c):
                x16s = []
                for s4 in range(4):
                    xt32 = xin.tile([P, D], F32, name=f"xc32_{c}_{s4}", tag="xin")
                    nc.scalar.dma_start(xt32[:], xc[c * CH + s4 * P:c * CH + (s4 + 1) * P, :])
                    xt16 = x16t.tile([P, D], F16, name=f"xc16_{c}_{s4}", tag="x16")
                    nc.vector.tensor_copy(xt16[:], xt32[:])
                    x16s.append(xt16)
                return x16s

            def load_cast_xd(t):
                xt32 = xdinp.tile([P, D], F32, name=f"xd32_{t}", tag="xdin")
                nc.sync.dma_start(xt32[:], xd[t * P:(t + 1) * P, :])
                nc.vector.tensor_copy(xdn[t][:], xt32[:])

            # chunk-0 xc casts first on DVE, then W_qc casts; xd loads
            # (sync queue) trickle casts into DVE gaps chunk by chunk
            x16s_c = {0: load_cast_xc(0)}
            for f in range(DB):
                w32 = winp.tile([P, D], F32, name=f"wq32_{f}", tag="win")
                nc.sync.dma_start(w32[:], wqc[f * P:(f + 1) * P, :])
                nc.vector.tensor_copy(wq16[f][:], w32[:])

            for c in range(NCH):
                csl = slice(c * CH, (c + 1) * CH)
                x16s = x16s_c.pop(c) if c in x16s_c else load_cast_xc(c)
                if c + 1 < NCH:
                    x16s_c[c + 1] = load_cast_xc(c + 1)
                for t in range(c * 4, c * 4 + 4):
                    load_cast_xd(t)
                for f in range(DB):
                    tp = tpsq.tile([P, CH], F16, name=f"tpq_{c}_{f}", tag="tp")
                    for s4 in range(4):
                        nc.tensor.transpose(tp[:, s4 * P:(s4 + 1) * P],
                                            x16s[s4][:, f * P:(f + 1) * P], ident[:])
                    nc.scalar.activation(xcT[f][:, csl], tp[:], Act.Copy)
                for e in range(DB):
                    pq = qps.tile([P, CH], F32, name=f"pq_{c}_{e}", tag="pq")
                    for f in range(DB):
                        nc.tensor.matmul(pq[:], wq16[f][:, e * P:(e + 1) * P],
                                         xcT[f][:, csl],
                                         start=(f == 0), stop=(f == DB - 1))
                    nc.scalar.activation(QT[e][:, csl], pq[:], Act.Copy)


        # ---- phase K: KT = W_kd^T @ xd^T ----
        with tc.tile_pool(name="xdT", bufs=1) as xdT_pool, \
             tc.tile_pool(name="wk16", bufs=1) as wk_pool, \
             tc.tile_pool(name="wkin", bufs=3) as wkin, \
             tc.tile_pool(name="tpsk", bufs=2, space="PSUM") as tpsk, \
             tc.tile_pool(name="kps", bufs=3, space="PSUM") as kps, \
             tc.tile_pool(name="dps", bufs=2, space="PSUM") as dps, \
             tc.tile_pool(name="kmisc", bufs=1) as kmisc:
            wk16 = [wk_pool.tile([P, D], F16, name=f"wk16_{f}") for f in range(DB)]
            for f in range(DB):
                w32 = wkin.tile([P, D], F32, name=f"wk32_{f}", tag="wkin")
                nc.scalar.dma_start(w32[:], wkd[f * P:(f + 1) * P, :])
                nc.vector.tensor_copy(wk16[f][:], w32[:])
            xdT = [xdT_pool.tile([P, SD], F16, name=f"xdT{f}") for f in range(DB)]
            for c in range(NCH):
                csl = slice(c * CH, (c + 1) * CH)
                for f in range(DB):
                    tp = tpsk.tile([P, CH], F16, name=f"tpk_{c}_{f}", tag="tp")
                    for s4 in range(4):
                        nc.tensor.transpose(tp[:, s4 * P:(s4 + 1) * P],
                                            xdn[c * 4 + s4][:, f * P:(f + 1) * P],
                                            ident[:])
                    nc.scalar.activation(xdT[f][:, csl], tp[:], Act.Copy)
                for e in range(DB):
                    pk = kps.tile([P, CH], F32, name=f"pk_{c}_{e}", tag="pk")
                    for f in range(DB):
                        nc.tensor.matmul(pk[:], wk16[f][:, e * P:(e + 1) * P],
                                         xdT[f][:, csl],
                                         start=(f == 0), stop=(f == DB - 1))
                    nc.vector.tensor_copy(KT[e][:, csl], pk[:])

            # xsum[d] = sum_s xc[s, d] ; Dout = xsum @ W_vc / (SD*32)  (fp32)
            xsum = kmisc.tile([P, DB], F32, name="xsum")
            for f in range(DB):
                nc.vector.tensor_reduce(xsum[:, f:f + 1], xcT[f][:], AxX, Alu.add)
            for ch in range(2):
                pd = dps.tile([1, CH], F32, name=f"pd_{ch}", tag="pd")
                for f in range(DB):
                    wv32 = wkin.tile([P, D], F32, name=f"wvc32_{ch}_{f}", tag="wkin")
                    nc.sync.dma_start(wv32[:], wvc[f * P:(f + 1) * P, :])
                    nc.tensor.matmul(pd[:], xsum[:, f:f + 1],
                                     wv32[:, ch * CH:(ch + 1) * CH],
                                     start=(f == 0), stop=(f == DB - 1))
                nc.scalar.activation(out_sb[0:1, D + ch * CH:D + (ch + 1) * CH], pd[:],
                                     Act.Copy, scale=1.0 / (SD * 32.0))

        xcT_scope.__exit__(None, None, None)

        # ---- phase L: logits + softmax column-sum ----
        tail = ctx.enter_context(tc.tile_pool(name="tail", bufs=1))
        wvd16 = [tail.tile([P, D], F16, name=f"wvd16_{f}") for f in range(DB)]

        with tc.tile_pool(name="lps", bufs=2, space="PSUM") as lps, \
             tc.tile_pool(name="epool", bufs=2) as epool, \
             tc.tile_pool(name="etpool", bufs=2) as etpool, \
             tc.tile_pool(name="wvdin", bufs=2) as wvdin, \
             tc.tile_pool(name="small", bufs=3) as small:
            for f in range(DB):
                wv32 = wvdin.tile([P, D], F32, name=f"wvd32_{f}", tag="wvdin")
                nc.scalar.dma_start(wv32[:], wvd[f * P:(f + 1) * P, :])
                nc.vector.tensor_copy(wvd16[f][:], wv32[:])
            for sb in range(SBK):
                L = lps.tile([P, SD], F32, name=f"L{sb}", tag="L")
                ssl = slice(sb * P, (sb + 1) * P)
                for c in range(NCH):
                    tsl = slice(c * CH, (c + 1) * CH)
                    for e in range(DB):
                        nc.tensor.matmul(L[:, tsl], QT[e][:, ssl], KT[e][:, tsl],
                                         start=(e == 0), stop=(e == DB - 1))
                mx = small.tile([P, 1], F32, name=f"mx{sb}", tag="mx")
                nc.vector.tensor_reduce(mx[:], L[:], AxX, Alu.max)
                negmx = small.tile([P, 1], F32, name=f"negmx{sb}", tag="negmx")
                nc.vector.tensor_scalar_mul(negmx[:], mx[:], -1.0)
                E = epool.tile([P, SD], F32, name=f"E{sb}", tag="E")
                Z = small.tile([P, 1], F32, name=f"Z{sb}", tag="Z")
                nc.scalar.activation(E[:], L[:], Act.Exp,
                                     bias=negmx[:], scale=1.0, accum_out=Z[:])
                rz = small.tile([P, 1], F32, name=f"rz{sb}", tag="rz")
                nc.vector.reciprocal(rz[:], Z[:])
                Et = etpool.tile([P, SD], F16, name=f"Et{sb}", tag="Et")
                nc.scalar.activation(Et[:], E[:], Act.Copy, scale=rz[:])
                nc.vector.tensor_add(cp[:], cp[:], Et[:])

        # ---- tail: w -> u -> C ----
        with tc.tile_pool(name="tps", bufs=2, space="PSUM") as tps, \
             tc.tile_pool(name="ups", bufs=2, space="PSUM") as ups, \
             tc.tile_pool(name="tmisc", bufs=1) as tmisc:
            wT = tmisc.tile([P, TB], F32, name="wT")
            for k in range(TB):
                tp = tps.tile([P, P], F16, name=f"tp{k}", tag="tp")
                nc.tensor.transpose(tp[:], cp[:, k * P:(k + 1) * P], ident[:])
                nc.vector.tensor_reduce(wT[:, k:k + 1], tp[:], AxX, Alu.add)
            nc.vector.tensor_copy(wT16[:], wT[:])

            # u = w @ x_d
            for ch in range(2):
                pu = ups.tile([1, CH], F32, name=f"pu{ch}", tag="pu")
                for t in range(TB):
                    nc.tensor.matmul(pu[:], wT16[:, t:t + 1],
                                     xdn[t][:, ch * CH:(ch + 1) * CH],
                                     start=(t == 0), stop=(t == TB - 1))
                nc.scalar.activation(u32[0:1, ch * CH:(ch + 1) * CH], pu[:], Act.Copy)

            # transpose u row -> uT16 columns
            put = tps.tile([P, DB], F32, name="put", tag="put")
            for e in range(DB):
                nc.tensor.transpose(put[:, e:e + 1], u32[0:1, e * P:(e + 1) * P],
                                    ones32[:])
            nc.scalar.activation(uT16[:], put[:], Act.Copy)

            # C = u @ W_vd / (SC*32)
            for ch in range(2):
                pv = ups.tile([1, CH], F32, name=f"pv{ch}", tag="pu")
                for f in range(DB):
                    nc.tensor.matmul(pv[:], uT16[:, f:f + 1],
                                     wvd16[f][:, ch * CH:(ch + 1) * CH],
                                     start=(f == 0), stop=(f == DB - 1))
                nc.scalar.activation(out_sb[0:1, ch * CH:(ch + 1) * CH], pv[:],
                                     Act.Copy, scale=1.0 / (SC * 32.0))
            nc.sync.dma_start(out_d, out_sb[:])

    _split_excess_waits(nc, mybir)
    return nc


def kernel(x_c, x_d, W_qc, W_vc, W_kd, W_vd):
    from concourse.bass_utils import run_bass_kernel_spmd
    nc = _build()
    in_maps = []
    for b in range(B):
        in_maps.append({
            "x_c": np.ascontiguousarray(x_c[b]),
            "x_d": np.ascontiguousarray(x_d[b]),
            "W_qc": np.asarray(W_qc), "W_vc": np.asarray(W_vc),
            "W_kd": np.asarray(W_kd), "W_vd": np.asarray(W_vd),
        })
    res = run_bass_kernel_spmd(nc, in_maps, list(range(B))).results
    C = np.empty((B, D), dtype=np.float32)
    Dout = np.empty((B, D), dtype=np.float32)
    for b in range(B):
        o = res[b]["out"]
        C[b] = o[0, :D]
        Dout[b] = o[0, D:]
    return (C, Dout)


# revision 17
# speedup vs baseline: 1.0428x; 1.0428x over previous
"""Trainium2 Bass kernel for nn_JointSelfAttentionLayer.

Math restructuring (both outputs are sequence-means):
  C    = (1/(SC*32)) * (w @ x_d) @ W_vd,   w[t] = sum_s softmax(logits)[s,t]
  Dout = (1/(SD*32)) * (sum_s x_c) @ W_vc  (softmax rows sum to 1)
so the heavy work is logits = (x_c@W_qc) @ (x_d@W_kd)^T plus a streaming
softmax column-sum. Never materializes Q, K, V_c, V_d, or A@V.

All big matmuls run single-pass f16 (1 cyc/row on the PE, fp32 PSUM
accumulation): end-to-end rel err ~1.6e-3 vs the 2e-2 gate. Inputs load
fp32 over the two HWDGE queues, cast f16 on DVE, transpose on the PE.
"""
import numpy as np
from contextlib import ExitStack

B, SC, SD, D = 8, 2048, 2048, 1024
P = 128
DB = D // P            # 8 d-blocks
CH = 512
NCH = SC // CH         # 4 s-chunks
SBK = SC // P          # 16 s-blocks
TB = SD // P           # 16 t-blocks


def _split_excess_waits(nc, mybir, max_waits=1):
    n = 0
    ctr = [0]
    for fn in nc.m.functions:
        for bb in fn.blocks:
            out = []
            changed = False
            for inst in bb.instructions:
                si = inst.sync_info
                ws = list(si.on_wait) if (si and si.on_wait) else []
                if len(ws) > max_waits and inst.engine != mybir.EngineType.Unassigned:
                    keep = ws[:max_waits]
                    excess = ws[max_waits:]
                    for i in range(0, len(excess), max_waits):
                        chunk = excess[i:i + max_waits]
                        nop = mybir.InstNoOp(name=f"ws_{ctr[0]}", ins=[], outs=[])
                        ctr[0] += 1
                        nop.engine = inst.engine
                        nop.sync_info = mybir.SyncInfo(on_wait=chunk, on_update=[])
                        out.append(nop)
                    inst.sync_info = mybir.SyncInfo(
                        on_wait=keep, on_update=list(si.on_update or []))
                    changed = True
                    n += 1
                out.append(inst)
            if changed:
                bb.instructions = out
    return n


def _build():
    import concourse.bass as bass
    import concourse.tile as tile
    from concourse import mybir
    from concourse.masks import make_identity

    F32 = mybir.dt.float32
    F16 = mybir.dt.float16
    Act = mybir.ActivationFunctionType
    Alu = mybir.AluOpType
    AxX = mybir.AxisListType.X

    nc = bass.Bass("TRN2", target_bir_lowering=False, debug=False, num_devices=8)
    xc = nc.dram_tensor("x_c", [SC, D], F32, kind="ExternalInput").ap()
    xd = nc.dram_tensor("x_d", [SD, D], F32, kind="ExternalInput").ap()
    wqc = nc.dram_tensor("W_qc", [D, D], F32, kind="ExternalInput").ap()
    wvc = nc.dram_tensor("W_vc", [D, D], F32, kind="ExternalInput").ap()
    wkd = nc.dram_tensor("W_kd", [D, D], F32, kind="ExternalInput").ap()
    wvd = nc.dram_tensor("W_vd", [D, D], F32, kind="ExternalInput").ap()
    out_d = nc.dram_tensor("out", [1, 2 * D], F32, kind="ExternalOutput").ap()

    with tile.TileContext(nc) as tc, ExitStack() as ctx:
        const = ctx.enter_context(tc.tile_pool(name="const", bufs=1))
        ident = const.tile([P, P], F16, name="ident")
        make_identity(nc, ident[:])
        cp = const.tile([P, SD], F16, name="cp")
        nc.gpsimd.memset(cp[:], 0.0)
        wT16 = const.tile([P, TB], F16, name="wT16")
        uT16 = const.tile([P, DB], F16, name="uT16")
        u32 = const.tile([1, D], F32, name="u32")
        ones32 = const.tile([1, 1], F32, name="ones32")
        nc.gpsimd.memset(ones32[:], 1.0)
        dummy = const.tile([1, 1], F32, name="dummy")
        out_sb = const.tile([1, 2 * D], F32, name="out_sb")

        qt_pool = ctx.enter_context(tc.tile_pool(name="qt", bufs=1))
        QT = [qt_pool.tile([P, SC], F16, name=f"QT{e}") for e in range(DB)]
        kt_pool = ctx.enter_context(tc.tile_pool(name="kt", bufs=1))
        KT = [kt_pool.tile([P, SD], F16, name=f"KT{e}") for e in range(DB)]
        xdn_pool = ctx.enter_context(tc.tile_pool(name="xdn", bufs=1))
        xdn = [xdn_pool.tile([P, D], F16, name=f"xdn{t}") for t in range(TB)]

        xcT_scope = tc.tile_pool(name="xcT", bufs=1)
        xcT_pool = xcT_scope.__enter__()

        # ---- phase Q: QT = W_qc^T @ xc^T ----
        with tc.tile_pool(name="xin", bufs=3) as xin, \
             tc.tile_pool(name="win", bufs=2) as winp, \
             tc.tile_pool(name="xdin", bufs=2) as xdinp, \
             tc.tile_pool(name="x16t", bufs=8) as x16t, \
             tc.tile_pool(name="wq16", bufs=1) as wq_pool, \
             tc.tile_pool(name="tpsq", bufs=3, space="PSUM") as tpsq, \
             tc.tile_pool(name="qps", bufs=4, space="PSUM") as qps:
            wq16 = [wq_pool.tile([P, D], F16, name=f"wq16_{f}") for f in range(DB)]
            xcT = [xcT_pool.tile([P, SC], F16, name=f"xcT{f}") for f in range(DB)]

            def load_cast_xc(c):
                x16s = []
                for s4 in range(4):
                    xt32 = xin.tile([P, D], F32, name=f"xc32_{c}_{s4}", tag="xin")
                    nc.sync.dma_start(xt32[:], xc[c * CH + s4 * P:c * CH + (s4 + 1) * P, :])
                    xt16 = x16t.tile([P, D], F16, name=f"xc16_{c}_{s4}", tag="x16")
                    nc.vector.tensor_copy(xt16[:], xt32[:])
                    x16s.append(xt16)
                return x16s

            def load_cast_xd(t):
                xt32 = xdinp.tile([P, D], F32, name=f"xd32_{t}", tag="xdin")
                nc.sync.dma_start(xt32[:], xd[t * P:(t + 1) * P, :])
                nc.vector.tensor_copy(xdn[t][:], xt32[:])

            # chunk-0 xc casts first on DVE, then W_qc casts; xd loads
            # (sync queue) trickle casts into DVE gaps chunk by chunk
            x16s_c = {0: load_cast_xc(0)}
            for f in range(DB):
                w32 = winp.tile([P, D], F32, name=f"wq32_{f}", tag="win")
                nc.scalar.dma_start(w32[:], wqc[f * P:(f + 1) * P, :])
                nc.vector.tensor_copy(wq16[f][:], w32[:])
            nc.scalar.activation(dummy[:], ones32[:], Act.Exp)

            for c in range(NCH):
                csl = slice(c * CH, (c + 1) * CH)
                x16s = x16s_c.pop(c) if c in x16s_c else load_cast_xc(c)
                if c + 1 < NCH:
                    x16s_c[c + 1] = load_cast_xc(c + 1)
                for t in range(c * 4, c * 4 + 4):
                    load_cast_xd(t)
                for f in range(DB):
                    tp = tpsq.tile([P, CH], F16, name=f"tpq_{c}_{f}", tag="tp")
                    for s4 in range(4):
                        nc.tensor.transpose(tp[:, s4 * P:(s4 + 1) * P],
                                            x16s[s4][:, f * P:(f + 1) * P], ident[:])
                    nc.scalar.activation(xcT[f][:, csl], tp[:], Act.Copy)
                for e in range(DB):
                    pq = qps.tile([P, CH], F32, name=f"pq_{c}_{e}", tag="pq")
                    for f in range(DB):
                        nc.tensor.matmul(pq[:], wq16[f][:, e * P:(e + 1) * P],
                                         xcT[f][:, csl],
                                         start=(f == 0), stop=(f == DB - 1))
                    nc.scalar.activation(QT[e][:, csl], pq[:], Act.Copy)


        # ---- phase K: KT = W_kd^T @ xd^T ----
        with tc.tile_pool(name="xdT", bufs=1) as xdT_pool, \
             tc.tile_pool(name="wk16", bufs=1) as wk_pool, \
             tc.tile_pool(name="wkin", bufs=3) as wkin, \
             tc.tile_pool(name="tpsk", bufs=2, space="PSUM") as tpsk, \
             tc.tile_pool(name="kps", bufs=3, space="PSUM") as kps, \
             tc.tile_pool(name="dps", bufs=2, space="PSUM") as dps, \
             tc.tile_pool(name="kmisc", bufs=1) as kmisc:
            wk16 = [wk_pool.tile([P, D], F16, name=f"wk16_{f}") for f in range(DB)]
            for f in range(DB):
                w32 = wkin.tile([P, D], F32, name=f"wk32_{f}", tag="wkin")
                nc.sync.dma_start(w32[:], wkd[f * P:(f + 1) * P, :])
                nc.vector.tensor_copy(wk16[f][:], w32[:])
            xdT = [xdT_pool.tile([P, SD], F16, name=f"xdT{f}") for f in range(DB)]
            for c in range(NCH):
                csl = slice(c * CH, (c + 1) * CH)
                for f in range(DB):
                    tp = tpsk.tile([P, CH], F16, name=f"tpk_{c}_{f}", tag="tp")
                    for s4 in range(4):
                        nc.tensor.transpose(tp[:, s4 * P:(s4 + 1) * P],
                                            xdn[c * 4 + s4][:, f * P:(f + 1) * P],
                                            ident[:])
                    nc.scalar.activation(xdT[f][:, csl], tp[:], Act.Copy)
                for e in range(DB):
                    pk = kps.tile([P, CH], F32, name=f"pk_{c}_{e}", tag="pk")
                    for f in range(DB):
                        nc.tensor.matmul(pk[:], wk16[f][:, e * P:(e + 1) * P],
                                         xdT[f][:, csl],
                                         start=(f == 0), stop=(f == DB - 1))
                    nc.vector.tensor_copy(KT[e][:, csl], pk[:])

            # xsum[d] = sum_s xc[s, d] ; Dout = xsum @ W_vc / (SD*32)  (fp32)
            xsum = kmisc.tile([P, DB], F32, name="xsum")
            for f in range(DB):
                nc.vector.tensor_reduce(xsum[:, f:f + 1], xcT[f][:], AxX, Alu.add)
            for ch in range(2):
                pd = dps.tile([1, CH], F32, name=f"pd_{ch}", tag="pd")
                for f in range(DB):
                    wv32 = wkin.tile([P, D], F32, name=f"wvc32_{ch}_{f}", tag="wkin")
                    nc.sync.dma_start(wv32[:], wvc[f * P:(f + 1) * P, :])
                    nc.tensor.matmul(pd[:], xsum[:, f:f + 1],
                                     wv32[:, ch * CH:(ch + 1) * CH],
                                     start=(f == 0), stop=(f == DB - 1))
                nc.scalar.activation(out_sb[0:1, D + ch * CH:D + (ch + 1) * CH], pd[:],
                                     Act.Copy, scale=1.0 / (SD * 32.0))

        xcT_scope.__exit__(None, None, None)

        # ---- phase L: logits + softmax column-sum ----
        tail = ctx.enter_context(tc.tile_pool(name="tail", bufs=1))
        wvd16 = [tail.tile([P, D], F16, name=f"wvd16_{f}") for f in range(DB)]

        with tc.tile_pool(name="lps", bufs=2, space="PSUM") as lps, \
             tc.tile_pool(name="epool", bufs=2) as epool, \
             tc.tile_pool(name="etpool", bufs=2) as etpool, \
             tc.tile_pool(name="wvdin", bufs=2) as wvdin, \
             tc.tile_pool(name="small", bufs=3) as small:
            for f in range(DB):
                wv32 = wvdin.tile([P, D], F32, name=f"wvd32_{f}", tag="wvdin")
                nc.scalar.dma_start(wv32[:], wvd[f * P:(f + 1) * P, :])
                nc.vector.tensor_copy(wvd16[f][:], wv32[:])
            for sb in range(SBK):
                L = lps.tile([P, SD], F32, name=f"L{sb}", tag="L")
                ssl = slice(sb * P, (sb + 1) * P)
                for c in range(NCH):
                    tsl = slice(c * CH, (c + 1) * CH)
                    for e in range(DB):
                        nc.tensor.matmul(L[:, tsl], QT[e][:, ssl], KT[e][:, tsl],
                                         start=(e == 0), stop=(e == DB - 1))
                mx = small.tile([P, 1], F32, name=f"mx{sb}", tag="mx")
                nc.vector.tensor_reduce(mx[:], L[:], AxX, Alu.max)
                negmx = small.tile([P, 1], F32, name=f"negmx{sb}", tag="negmx")
                nc.vector.tensor_scalar_mul(negmx[:], mx[:], -1.0)
                E = epool.tile([P, SD], F32, name=f"E{sb}", tag="E")
                Z = small.tile([P, 1], F32, name=f"Z{sb}", tag="Z")
                nc.scalar.activation(E[:], L[:], Act.Exp,
                                     bias=negmx[:], scale=1.0, accum_out=Z[:])
                rz = small.tile([P, 1], F32, name=f"rz{sb}", tag="rz")
                nc.vector.reciprocal(rz[:], Z[:])
                Et = etpool.tile([P, SD], F16, name=f"Et{sb}", tag="Et")
                nc.scalar.activation(Et[:], E[:], Act.Copy, scale=rz[:])
                nc.vector.tensor_add(cp[:], cp[:], Et[:])

        # ---- tail: w -> u -> C ----
        with tc.tile_pool(name="tps", bufs=2, space="PSUM") as tps, \
             tc.tile_pool(name="ups", bufs=2, space="PSUM") as ups, \
             tc.tile_pool(name="tmisc", bufs=1) as tmisc:
            wT = tmisc.tile([P, TB], F32, name="wT")
            for k in range(TB):
                tp = tps.tile([P, P], F16, name=f"tp{k}", tag="tp")
                nc.tensor.transpose(tp[:], cp[:, k * P:(k + 1) * P], ident[:])
                nc.vector.tensor_reduce(wT[:, k:k + 1], tp[:], AxX, Alu.add)
            nc.vector.tensor_copy(wT16[:], wT[:])

            # u = w @ x_d
            for ch in range(2):
                pu = ups.tile([1, CH], F32, name=f"pu{ch}", tag="pu")
                for t in range(TB):
                    nc.tensor.matmul(pu[:], wT16[:, t:t + 1],
                                     xdn[t][:, ch * CH:(ch + 1) * CH],
                                     start=(t == 0), stop=(t == TB - 1))
                nc.scalar.activation(u32[0:1, ch * CH:(ch + 1) * CH], pu[:], Act.Copy)

            # transpose u row -> uT16 columns
            put = tps.tile([P, DB], F32, name="put", tag="put")
            for e in range(DB):
                nc.tensor.transpose(put[:, e:e + 1], u32[0:1, e * P:(e + 1) * P],
                                    ones32[:])
            nc.scalar.activation(uT16[:], put[:], Act.Copy)

            # C = u @ W_vd / (SC*32)
            for ch in range(2):
                pv = ups.tile([1, CH], F32, name=f"pv{ch}", tag="pu")
                for f in range(DB):
                    nc.tensor.matmul(pv[:], uT16[:, f:f + 1],
                                     wvd16[f][:, ch * CH:(ch + 1) * CH],
                                     start=(f == 0), stop=(f == DB - 1))
                nc.scalar.activation(out_sb[0:1, ch * CH:(ch + 1) * CH], pv[:],
                                     Act.Copy, scale=1.0 / (SC * 32.0))
            nc.sync.dma_start(out_d, out_sb[:])

    _split_excess_waits(nc, mybir)
    return nc


def kernel(x_c, x_d, W_qc, W_vc, W_kd, W_vd):
    from concourse.bass_utils import run_bass_kernel_spmd
    nc = _build()
    in_maps = []
    for b in range(B):
        in_maps.append({
            "x_c": np.ascontiguousarray(x_c[b]),
            "x_d": np.ascontiguousarray(x_d[b]),
            "W_qc": np.asarray(W_qc), "W_vc": np.asarray(W_vc),
            "W_kd": np.asarray(W_kd), "W_vd": np.asarray(W_vd),
        })
    res = run_bass_kernel_spmd(nc, in_maps, list(range(B))).results
    C = np.empty((B, D), dtype=np.float32)
    Dout = np.empty((B, D), dtype=np.float32)
    for b in range(B):
        o = res[b]["out"]
        C[b] = o[0, :D]
        Dout[b] = o[0, D:]
    return (C, Dout)


# revision 18
# speedup vs baseline: 1.0634x; 1.0198x over previous
"""Trainium2 Bass kernel for nn_JointSelfAttentionLayer.

Math restructuring (both outputs are sequence-means):
  C    = (1/(SC*32)) * (w @ x_d) @ W_vd,   w[t] = sum_s softmax(logits)[s,t]
  Dout = (1/(SD*32)) * (sum_s x_c) @ W_vc  (softmax rows sum to 1)
so the heavy work is logits = (x_c@W_qc) @ (x_d@W_kd)^T plus a streaming
softmax column-sum. Never materializes Q, K, V_c, V_d, or A@V.

All big matmuls run single-pass f16 (1 cyc/row on the PE, fp32 PSUM
accumulation): end-to-end rel err ~1.6e-3 vs the 2e-2 gate. Inputs load
fp32 over the two HWDGE queues, cast f16 on DVE, transpose on the PE.
"""
import numpy as np
from contextlib import ExitStack

B, SC, SD, D = 8, 2048, 2048, 1024
P = 128
DB = D // P            # 8 d-blocks
CH = 512
NCH = SC // CH         # 4 s-chunks
SBK = SC // P          # 16 s-blocks
TB = SD // P           # 16 t-blocks


def _split_excess_waits(nc, mybir, max_waits=1):
    n = 0
    ctr = [0]
    for fn in nc.m.functions:
        for bb in fn.blocks:
            out = []
            changed = False
            for inst in bb.instructions:
                si = inst.sync_info
                ws = list(si.on_wait) if (si and si.on_wait) else []
                if len(ws) > max_waits and inst.engine != mybir.EngineType.Unassigned:
                    keep = ws[:max_waits]
                    excess = ws[max_waits:]
                    for i in range(0, len(excess), max_waits):
                        chunk = excess[i:i + max_waits]
                        nop = mybir.InstNoOp(name=f"ws_{ctr[0]}", ins=[], outs=[])
                        ctr[0] += 1
                        nop.engine = inst.engine
                        nop.sync_info = mybir.SyncInfo(on_wait=chunk, on_update=[])
                        out.append(nop)
                    inst.sync_info = mybir.SyncInfo(
                        on_wait=keep, on_update=list(si.on_update or []))
                    changed = True
                    n += 1
                out.append(inst)
            if changed:
                bb.instructions = out
    return n


def _build():
    import concourse.bass as bass
    import concourse.tile as tile
    from concourse import mybir
    from concourse.masks import make_identity

    F32 = mybir.dt.float32
    F16 = mybir.dt.float16
    Act = mybir.ActivationFunctionType
    Alu = mybir.AluOpType
    AxX = mybir.AxisListType.X

    nc = bass.Bass("TRN2", target_bir_lowering=False, debug=False, num_devices=8)
    xc = nc.dram_tensor("x_c", [SC, D], F32, kind="ExternalInput").ap()
    xd = nc.dram_tensor("x_d", [SD, D], F32, kind="ExternalInput").ap()
    wqc = nc.dram_tensor("W_qc", [D, D], F32, kind="ExternalInput").ap()
    wvc = nc.dram_tensor("W_vc", [D, D], F32, kind="ExternalInput").ap()
    wkd = nc.dram_tensor("W_kd", [D, D], F32, kind="ExternalInput").ap()
    wvd = nc.dram_tensor("W_vd", [D, D], F32, kind="ExternalInput").ap()
    out_d = nc.dram_tensor("out", [1, 2 * D], F32, kind="ExternalOutput").ap()

    with tile.TileContext(nc) as tc, ExitStack() as ctx:
        const = ctx.enter_context(tc.tile_pool(name="const", bufs=1))
        ident = const.tile([P, P], F16, name="ident")
        make_identity(nc, ident[:])
        cp = const.tile([P, SD], F16, name="cp")
        nc.gpsimd.memset(cp[:], 0.0)
        wT16 = const.tile([P, TB], F16, name="wT16")
        uT16 = const.tile([P, DB], F16, name="uT16")
        u32 = const.tile([1, D], F32, name="u32")
        ones32 = const.tile([1, 1], F32, name="ones32")
        nc.gpsimd.memset(ones32[:], 1.0)
        dummy = const.tile([1, 1], F32, name="dummy")
        out_sb = const.tile([1, 2 * D], F32, name="out_sb")

        qt_pool = ctx.enter_context(tc.tile_pool(name="qt", bufs=1))
        QT = [qt_pool.tile([P, SC], F16, name=f"QT{e}") for e in range(DB)]
        kt_pool = ctx.enter_context(tc.tile_pool(name="kt", bufs=1))
        KT = [kt_pool.tile([P, SD], F16, name=f"KT{e}") for e in range(DB)]
        xdn_pool = ctx.enter_context(tc.tile_pool(name="xdn", bufs=1))
        xdn = [xdn_pool.tile([P, D], F16, name=f"xdn{t}") for t in range(TB)]

        xcT_scope = tc.tile_pool(name="xcT", bufs=1)
        xcT_pool = xcT_scope.__enter__()

        # ---- phase Q: QT = W_qc^T @ xc^T ----
        with tc.tile_pool(name="xin", bufs=3) as xin, \
             tc.tile_pool(name="win", bufs=2) as winp, \
             tc.tile_pool(name="xdin", bufs=2) as xdinp, \
             tc.tile_pool(name="x16t", bufs=8) as x16t, \
             tc.tile_pool(name="wq16", bufs=1) as wq_pool, \
             tc.tile_pool(name="tpsq", bufs=3, space="PSUM") as tpsq, \
             tc.tile_pool(name="qps", bufs=4, space="PSUM") as qps:
            wq16 = [wq_pool.tile([P, D], F16, name=f"wq16_{f}") for f in range(DB)]
            xcT = [xcT_pool.tile([P, SC], F16, name=f"xcT{f}") for f in range(DB)]

            def load_cast_xc(c):
                x16s = []
                for s4 in range(4):
                    xt32 = xin.tile([P, D], F32, name=f"xc32_{c}_{s4}", tag="xin")
                    nc.sync.dma_start(xt32[:], xc[c * CH + s4 * P:c * CH + (s4 + 1) * P, :])
                    xt16 = x16t.tile([P, D], F16, name=f"xc16_{c}_{s4}", tag="x16")
                    nc.vector.tensor_copy(xt16[:], xt32[:])
                    x16s.append(xt16)
                return x16s

            def load_cast_xd(t):
                xt32 = xdinp.tile([P, D], F32, name=f"xd32_{t}", tag="xdin")
                nc.sync.dma_start(xt32[:], xd[t * P:(t + 1) * P, :])
                nc.vector.tensor_copy(xdn[t][:], xt32[:])

            # chunk-0 xc casts first on DVE, then W_qc casts; xd loads
            # (sync queue) trickle casts into DVE gaps chunk by chunk
            x16s_c = {0: load_cast_xc(0)}
            for f in range(DB):
                w32 = winp.tile([P, D], F32, name=f"wq32_{f}", tag="win")
                nc.scalar.dma_start(w32[:], wqc[f * P:(f + 1) * P, :])
                nc.vector.tensor_copy(wq16[f][:], w32[:])
            nc.scalar.activation(dummy[:], ones32[:], Act.Exp)

            for c in range(NCH):
                csl = slice(c * CH, (c + 1) * CH)
                x16s = x16s_c.pop(c) if c in x16s_c else load_cast_xc(c)
                if c + 1 < NCH:
                    x16s_c[c + 1] = load_cast_xc(c + 1)
                for t in range(c * 4, c * 4 + 4):
                    load_cast_xd(t)
                for f in range(DB):
                    tp = tpsq.tile([P, CH], F16, name=f"tpq_{c}_{f}", tag="tp")
                    for s4 in range(4):
                        nc.tensor.transpose(tp[:, s4 * P:(s4 + 1) * P],
                                            x16s[s4][:, f * P:(f + 1) * P], ident[:])
                    nc.scalar.activation(xcT[f][:, csl], tp[:], Act.Copy)
                for e in range(DB):
                    pq = qps.tile([P, CH], F32, name=f"pq_{c}_{e}", tag="pq")
                    for f in range(DB):
                        nc.tensor.matmul(pq[:], wq16[f][:, e * P:(e + 1) * P],
                                         xcT[f][:, csl],
                                         start=(f == 0), stop=(f == DB - 1))
                    nc.scalar.activation(QT[e][:, csl], pq[:], Act.Copy)


        # ---- phase K: KT = W_kd^T @ xd^T ----
        with tc.tile_pool(name="xdT", bufs=1) as xdT_pool, \
             tc.tile_pool(name="wk16", bufs=1) as wk_pool, \
             tc.tile_pool(name="wkin", bufs=3) as wkin, \
             tc.tile_pool(name="tpsk", bufs=2, space="PSUM") as tpsk, \
             tc.tile_pool(name="kps", bufs=4, space="PSUM") as kps, \
             tc.tile_pool(name="dps", bufs=2, space="PSUM") as dps, \
             tc.tile_pool(name="kmisc", bufs=1) as kmisc:
            wk16 = [wk_pool.tile([P, D], F16, name=f"wk16_{f}") for f in range(DB)]
            for f in range(DB):
                w32 = wkin.tile([P, D], F32, name=f"wk32_{f}", tag="wkin")
                nc.sync.dma_start(w32[:], wkd[f * P:(f + 1) * P, :])
                nc.vector.tensor_copy(wk16[f][:], w32[:])
            xdT = [xdT_pool.tile([P, SD], F16, name=f"xdT{f}") for f in range(DB)]
            for c in range(NCH):
                csl = slice(c * CH, (c + 1) * CH)
                for f in range(DB):
                    tp = tpsk.tile([P, CH], F16, name=f"tpk_{c}_{f}", tag="tp")
                    for s4 in range(4):
                        nc.tensor.transpose(tp[:, s4 * P:(s4 + 1) * P],
                                            xdn[c * 4 + s4][:, f * P:(f + 1) * P],
                                            ident[:])
                    nc.scalar.activation(xdT[f][:, csl], tp[:], Act.Copy)
                for e in range(DB):
                    pk = kps.tile([P, CH], F32, name=f"pk_{c}_{e}", tag="pk")
                    for f in range(DB):
                        nc.tensor.matmul(pk[:], wk16[f][:, e * P:(e + 1) * P],
                                         xdT[f][:, csl],
                                         start=(f == 0), stop=(f == DB - 1))
                    nc.scalar.activation(KT[e][:, csl], pk[:], Act.Copy)

            # xsum[d] = sum_s xc[s, d] ; Dout = xsum @ W_vc / (SD*32)  (fp32)
            xsum = kmisc.tile([P, DB], F32, name="xsum")
            for f in range(DB):
                nc.vector.tensor_reduce(xsum[:, f:f + 1], xcT[f][:], AxX, Alu.add)
            for ch in range(2):
                pd = dps.tile([1, CH], F32, name=f"pd_{ch}", tag="pd")
                for f in range(DB):
                    wv32 = wkin.tile([P, D], F32, name=f"wvc32_{ch}_{f}", tag="wkin")
                    nc.sync.dma_start(wv32[:], wvc[f * P:(f + 1) * P, :])
                    nc.tensor.matmul(pd[:], xsum[:, f:f + 1],
                                     wv32[:, ch * CH:(ch + 1) * CH],
                                     start=(f == 0), stop=(f == DB - 1))
                nc.scalar.activation(out_sb[0:1, D + ch * CH:D + (ch + 1) * CH], pd[:],
                                     Act.Copy, scale=1.0 / (SD * 32.0))

        xcT_scope.__exit__(None, None, None)

        # ---- phase L: logits + softmax column-sum ----
        tail = ctx.enter_context(tc.tile_pool(name="tail", bufs=1))
        wvd16 = [tail.tile([P, D], F16, name=f"wvd16_{f}") for f in range(DB)]

        with tc.tile_pool(name="lps", bufs=2, space="PSUM") as lps, \
             tc.tile_pool(name="epool", bufs=2) as epool, \
             tc.tile_pool(name="etpool", bufs=2) as etpool, \
             tc.tile_pool(name="wvdin", bufs=2) as wvdin, \
             tc.tile_pool(name="small", bufs=3) as small:
            for sb in range(SBK):
                if 4 <= sb < 4 + DB:
                    f = sb - 4
                    wv32 = wvdin.tile([P, D], F32, name=f"wvd32_{f}", tag="wvdin")
                    nc.scalar.dma_start(wv32[:], wvd[f * P:(f + 1) * P, :])
                    nc.vector.tensor_copy(wvd16[f][:], wv32[:])
                L = lps.tile([P, SD], F32, name=f"L{sb}", tag="L")
                ssl = slice(sb * P, (sb + 1) * P)
                for c in range(NCH):
                    tsl = slice(c * CH, (c + 1) * CH)
                    for e in range(DB):
                        nc.tensor.matmul(L[:, tsl], QT[e][:, ssl], KT[e][:, tsl],
                                         start=(e == 0), stop=(e == DB - 1))
                mx = small.tile([P, 1], F32, name=f"mx{sb}", tag="mx")
                nc.vector.tensor_reduce(mx[:], L[:], AxX, Alu.max)
                negmx = small.tile([P, 1], F32, name=f"negmx{sb}", tag="negmx")
                nc.vector.tensor_scalar_mul(negmx[:], mx[:], -1.0)
                E = epool.tile([P, SD], F32, name=f"E{sb}", tag="E")
                Z = small.tile([P, 1], F32, name=f"Z{sb}", tag="Z")
                nc.scalar.activation(E[:], L[:], Act.Exp,
                                     bias=negmx[:], scale=1.0, accum_out=Z[:])
                rz = small.tile([P, 1], F32, name=f"rz{sb}", tag="rz")
                nc.vector.reciprocal(rz[:], Z[:])
                Et = etpool.tile([P, SD], F16, name=f"Et{sb}", tag="Et")
                nc.scalar.activation(Et[:], E[:], Act.Copy, scale=rz[:])
                nc.vector.tensor_add(cp[:], cp[:], Et[:])

        # ---- tail: w -> u -> C ----
        with tc.tile_pool(name="tps", bufs=2, space="PSUM") as tps, \
             tc.tile_pool(name="ups", bufs=2, space="PSUM") as ups, \
             tc.tile_pool(name="tmisc", bufs=1) as tmisc:
            wT = tmisc.tile([P, TB], F32, name="wT")
            for k in range(TB):
                tp = tps.tile([P, P], F16, name=f"tp{k}", tag="tp")
                nc.tensor.transpose(tp[:], cp[:, k * P:(k + 1) * P], ident[:])
                nc.vector.tensor_reduce(wT[:, k:k + 1], tp[:], AxX, Alu.add)
            nc.vector.tensor_copy(wT16[:], wT[:])

            # u = w @ x_d
            for ch in range(2):
                pu = ups.tile([1, CH], F32, name=f"pu{ch}", tag="pu")
                for t in range(TB):
                    nc.tensor.matmul(pu[:], wT16[:, t:t + 1],
                                     xdn[t][:, ch * CH:(ch + 1) * CH],
                                     start=(t == 0), stop=(t == TB - 1))
                if ch == 0:
                    nc.scalar.activation(u32[0:1, ch * CH:(ch + 1) * CH], pu[:], Act.Copy)
                else:
                    nc.vector.tensor_copy(u32[0:1, ch * CH:(ch + 1) * CH], pu[:])

            # transpose u row -> uT16 columns
            put = tps.tile([P, DB], F32, name="put", tag="put")
            for e in range(DB):
                nc.tensor.transpose(put[:, e:e + 1], u32[0:1, e * P:(e + 1) * P],
                                    ones32[:])
            nc.scalar.activation(uT16[:], put[:], Act.Copy)

            # C = u @ W_vd / (SC*32)
            for ch in range(2):
                pv = ups.tile([1, CH], F32, name=f"pv{ch}", tag="pu")
                for f in range(DB):
                    nc.tensor.matmul(pv[:], uT16[:, f:f + 1],
                                     wvd16[f][:, ch * CH:(ch + 1) * CH],
                                     start=(f == 0), stop=(f == DB - 1))
                if ch == 0:
                    nc.scalar.activation(out_sb[0:1, ch * CH:(ch + 1) * CH], pv[:],
                                         Act.Copy, scale=1.0 / (SC * 32.0))
                else:
                    nc.vector.tensor_scalar_mul(out_sb[0:1, ch * CH:(ch + 1) * CH],
                                                pv[:], 1.0 / (SC * 32.0))
            nc.sync.dma_start(out_d, out_sb[:])

    _split_excess_waits(nc, mybir)
    return nc


def kernel(x_c, x_d, W_qc, W_vc, W_kd, W_vd):
    from concourse.bass_utils import run_bass_kernel_spmd
    nc = _build()
    in_maps = []
    for b in range(B):
        in_maps.append({
            "x_c": np.ascontiguousarray(x_c[b]),
            "x_d": np.ascontiguousarray(x_d[b]),
            "W_qc": np.asarray(W_qc), "W_vc": np.asarray(W_vc),
            "W_kd": np.asarray(W_kd), "W_vd": np.asarray(W_vd),
        })
    res = run_bass_kernel_spmd(nc, in_maps, list(range(B))).results
    C = np.empty((B, D), dtype=np.float32)
    Dout = np.empty((B, D), dtype=np.float32)
    for b in range(B):
        o = res[b]["out"]
        C[b] = o[0, :D]
        Dout[b] = o[0, D:]
    return (C, Dout)


# revision 19
# speedup vs baseline: 1.1198x; 1.0530x over previous
"""Trainium2 Bass kernel for nn_JointSelfAttentionLayer.

Math restructuring (both outputs are sequence-means):
  C    = (1/(SC*32)) * (w @ x_d) @ W_vd,   w[t] = sum_s softmax(logits)[s,t]
  Dout = (1/(SD*32)) * (sum_s x_c) @ W_vc  (softmax rows sum to 1)
so the heavy work is logits = (x_c@W_qc) @ (x_d@W_kd)^T plus a streaming
softmax column-sum. Never materializes Q, K, V_c, V_d, or A@V.

All big matmuls run single-pass f16 (1 cyc/row on the PE, fp32 PSUM
accumulation): end-to-end rel err ~1.6e-3 vs the 2e-2 gate. Inputs load
fp32 over the two HWDGE queues, cast f16 on DVE, transpose on the PE.
"""
import numpy as np
from contextlib import ExitStack

B, SC, SD, D = 8, 2048, 2048, 1024
P = 128
DB = D // P            # 8 d-blocks
CH = 512
NCH = SC // CH         # 4 s-chunks
SBK = SC // P          # 16 s-blocks
TB = SD // P           # 16 t-blocks


def _split_excess_waits(nc, mybir, max_waits=1):
    n = 0
    ctr = [0]
    for fn in nc.m.functions:
        for bb in fn.blocks:
            out = []
            changed = False
            for inst in bb.instructions:
                si = inst.sync_info
                ws = list(si.on_wait) if (si and si.on_wait) else []
                if len(ws) > max_waits and inst.engine != mybir.EngineType.Unassigned:
                    keep = ws[:max_waits]
                    excess = ws[max_waits:]
                    for i in range(0, len(excess), max_waits):
                        chunk = excess[i:i + max_waits]
                        nop = mybir.InstNoOp(name=f"ws_{ctr[0]}", ins=[], outs=[])
                        ctr[0] += 1
                        nop.engine = inst.engine
                        nop.sync_info = mybir.SyncInfo(on_wait=chunk, on_update=[])
                        out.append(nop)
                    inst.sync_info = mybir.SyncInfo(
                        on_wait=keep, on_update=list(si.on_update or []))
                    changed = True
                    n += 1
                out.append(inst)
            if changed:
                bb.instructions = out
    return n


def _build():
    import concourse.bass as bass
    import concourse.tile as tile
    from concourse import mybir
    from concourse.masks import make_identity

    F32 = mybir.dt.float32
    F16 = mybir.dt.float16
    Act = mybir.ActivationFunctionType
    Alu = mybir.AluOpType
    AxX = mybir.AxisListType.X

    nc = bass.Bass("TRN2", target_bir_lowering=False, debug=False, num_devices=8)
    xc = nc.dram_tensor("x_c", [SC, D], F32, kind="ExternalInput").ap()
    xd = nc.dram_tensor("x_d", [SD, D], F32, kind="ExternalInput").ap()
    wqc = nc.dram_tensor("W_qc", [D, D], F32, kind="ExternalInput").ap()
    wvc = nc.dram_tensor("W_vc", [D, D], F32, kind="ExternalInput").ap()
    wkd = nc.dram_tensor("W_kd", [D, D], F32, kind="ExternalInput").ap()
    wvd = nc.dram_tensor("W_vd", [D, D], F32, kind="ExternalInput").ap()
    out_d = nc.dram_tensor("out", [1, 2 * D], F32, kind="ExternalOutput").ap()

    with tile.TileContext(nc) as tc, ExitStack() as ctx:
        const = ctx.enter_context(tc.tile_pool(name="const", bufs=1))
        ident = const.tile([P, P], F16, name="ident")
        make_identity(nc, ident[:])
        cp = const.tile([P, SD], F16, name="cp")
        nc.gpsimd.memset(cp[:], 0.0)
        wT16 = const.tile([P, TB], F16, name="wT16")
        uT16 = const.tile([P, DB], F16, name="uT16")
        u32 = const.tile([1, D], F32, name="u32")
        ones32 = const.tile([1, 1], F32, name="ones32")
        nc.gpsimd.memset(ones32[:], 1.0)
        dummy = const.tile([1, 1], F32, name="dummy")
        out_sb = const.tile([1, 2 * D], F32, name="out_sb")
        xsum = const.tile([P, DB], F32, name="xsum")

        qt_pool = ctx.enter_context(tc.tile_pool(name="qt", bufs=1))
        QT = [qt_pool.tile([P, SC], F16, name=f"QT{e}") for e in range(DB)]
        kt_pool = ctx.enter_context(tc.tile_pool(name="kt", bufs=1))
        KT = [kt_pool.tile([P, SD], F16, name=f"KT{e}") for e in range(DB)]
        xdn_pool = ctx.enter_context(tc.tile_pool(name="xdn", bufs=1))
        xdn = [xdn_pool.tile([P, D], F16, name=f"xdn{t}") for t in range(TB)]

        xcT_scope = tc.tile_pool(name="xcT", bufs=1)
        xcT_pool = xcT_scope.__enter__()

        # ---- phase Q: QT = W_qc^T @ xc^T ----
        with tc.tile_pool(name="xin", bufs=3) as xin, \
             tc.tile_pool(name="win", bufs=2) as winp, \
             tc.tile_pool(name="xdin", bufs=2) as xdinp, \
             tc.tile_pool(name="x16t", bufs=8) as x16t, \
             tc.tile_pool(name="wq16", bufs=1) as wq_pool, \
             tc.tile_pool(name="tpsq", bufs=3, space="PSUM") as tpsq, \
             tc.tile_pool(name="qps", bufs=4, space="PSUM") as qps:
            wq16 = [wq_pool.tile([P, D], F16, name=f"wq16_{f}") for f in range(DB)]
            xcT = [xcT_pool.tile([P, SC], F16, name=f"xcT{f}") for f in range(DB)]

            def load_cast_xc(c):
                x16s = []
                for s4 in range(4):
                    xt32 = xin.tile([P, D], F32, name=f"xc32_{c}_{s4}", tag="xin")
                    nc.sync.dma_start(xt32[:], xc[c * CH + s4 * P:c * CH + (s4 + 1) * P, :])
                    xt16 = x16t.tile([P, D], F16, name=f"xc16_{c}_{s4}", tag="x16")
                    nc.vector.tensor_copy(xt16[:], xt32[:])
                    x16s.append(xt16)
                return x16s

            def load_cast_xd(t):
                xt32 = xdinp.tile([P, D], F32, name=f"xd32_{t}", tag="xdin")
                nc.sync.dma_start(xt32[:], xd[t * P:(t + 1) * P, :])
                nc.vector.tensor_copy(xdn[t][:], xt32[:])

            # chunk-0 xc casts first on DVE, then W_qc casts; xd loads
            # (sync queue) trickle casts into DVE gaps chunk by chunk
            x16s_c = {0: load_cast_xc(0)}
            for f in range(DB):
                w32 = winp.tile([P, D], F32, name=f"wq32_{f}", tag="win")
                nc.scalar.dma_start(w32[:], wqc[f * P:(f + 1) * P, :])
                nc.scalar.activation(wq16[f][:], w32[:], Act.Copy)
            nc.scalar.activation(dummy[:], ones32[:], Act.Exp)

            for c in range(NCH):
                csl = slice(c * CH, (c + 1) * CH)
                x16s = x16s_c.pop(c) if c in x16s_c else load_cast_xc(c)
                if c + 1 < NCH:
                    x16s_c[c + 1] = load_cast_xc(c + 1)
                for t in range(c * 4, c * 4 + 4):
                    load_cast_xd(t)
                for f in range(DB):
                    tp = tpsq.tile([P, CH], F16, name=f"tpq_{c}_{f}", tag="tp")
                    for s4 in range(4):
                        nc.tensor.transpose(tp[:, s4 * P:(s4 + 1) * P],
                                            x16s[s4][:, f * P:(f + 1) * P], ident[:])
                    nc.scalar.activation(xcT[f][:, csl], tp[:], Act.Copy)
                for e in range(DB):
                    pq = qps.tile([P, CH], F32, name=f"pq_{c}_{e}", tag="pq")
                    for f in range(DB):
                        nc.tensor.matmul(pq[:], wq16[f][:, e * P:(e + 1) * P],
                                         xcT[f][:, csl],
                                         start=(f == 0), stop=(f == DB - 1))
                    nc.scalar.activation(QT[e][:, csl], pq[:], Act.Copy)


        # ---- phase K: KT = W_kd^T @ xd^T ----
        with tc.tile_pool(name="xdT", bufs=1) as xdT_pool, \
             tc.tile_pool(name="wk16", bufs=1) as wk_pool, \
             tc.tile_pool(name="wkin", bufs=3) as wkin, \
             tc.tile_pool(name="tpsk", bufs=2, space="PSUM") as tpsk, \
             tc.tile_pool(name="kps", bufs=4, space="PSUM") as kps:
            wk16 = [wk_pool.tile([P, D], F16, name=f"wk16_{f}") for f in range(DB)]
            for f in range(DB):
                w32 = wkin.tile([P, D], F32, name=f"wk32_{f}", tag="wkin")
                nc.sync.dma_start(w32[:], wkd[f * P:(f + 1) * P, :])
                nc.vector.tensor_copy(wk16[f][:], w32[:])
            xdT = [xdT_pool.tile([P, SD], F16, name=f"xdT{f}") for f in range(DB)]
            for c in range(NCH):
                csl = slice(c * CH, (c + 1) * CH)
                for f in range(DB):
                    tp = tpsk.tile([P, CH], F16, name=f"tpk_{c}_{f}", tag="tp")
                    for s4 in range(4):
                        nc.tensor.transpose(tp[:, s4 * P:(s4 + 1) * P],
                                            xdn[c * 4 + s4][:, f * P:(f + 1) * P],
                                            ident[:])
                    nc.scalar.activation(xdT[f][:, csl], tp[:], Act.Copy)
                for e in range(DB):
                    pk = kps.tile([P, CH], F32, name=f"pk_{c}_{e}", tag="pk")
                    for f in range(DB):
                        nc.tensor.matmul(pk[:], wk16[f][:, e * P:(e + 1) * P],
                                         xdT[f][:, csl],
                                         start=(f == 0), stop=(f == DB - 1))
                    nc.scalar.activation(KT[e][:, csl], pk[:], Act.Copy)

            # xsum[d] = sum_s xc[s, d]
            for f in range(DB):
                nc.vector.tensor_reduce(xsum[:, f:f + 1], xcT[f][:], AxX, Alu.add)

        xcT_scope.__exit__(None, None, None)

        # ---- phase L: logits + softmax column-sum ----
        tail = ctx.enter_context(tc.tile_pool(name="tail", bufs=1))
        wvd16 = [tail.tile([P, D], F16, name=f"wvd16_{f}") for f in range(DB)]
        wvc32 = [tail.tile([P, D], F32, name=f"wvc32_{f}") for f in range(DB)]

        with tc.tile_pool(name="lps", bufs=2, space="PSUM") as lps, \
             tc.tile_pool(name="epool", bufs=2) as epool, \
             tc.tile_pool(name="etpool", bufs=2) as etpool, \
             tc.tile_pool(name="wvdin", bufs=2) as wvdin, \
             tc.tile_pool(name="small", bufs=3) as small:
            for sb in range(SBK):
                if sb < DB:
                    nc.sync.dma_start(wvc32[sb][:], wvc[sb * P:(sb + 1) * P, :])
                if 4 <= sb < 4 + DB:
                    f = sb - 4
                    wv32 = wvdin.tile([P, D], F32, name=f"wvd32_{f}", tag="wvdin")
                    nc.scalar.dma_start(wv32[:], wvd[f * P:(f + 1) * P, :])
                    nc.vector.tensor_copy(wvd16[f][:], wv32[:])
                L = lps.tile([P, SD], F32, name=f"L{sb}", tag="L")
                ssl = slice(sb * P, (sb + 1) * P)
                for c in range(NCH):
                    tsl = slice(c * CH, (c + 1) * CH)
                    for e in range(DB):
                        nc.tensor.matmul(L[:, tsl], QT[e][:, ssl], KT[e][:, tsl],
                                         start=(e == 0), stop=(e == DB - 1))
                mx = small.tile([P, 1], F32, name=f"mx{sb}", tag="mx")
                nc.vector.tensor_reduce(mx[:], L[:], AxX, Alu.max)
                negmx = small.tile([P, 1], F32, name=f"negmx{sb}", tag="negmx")
                nc.vector.tensor_scalar_mul(negmx[:], mx[:], -1.0)
                E = epool.tile([P, SD], F32, name=f"E{sb}", tag="E")
                Z = small.tile([P, 1], F32, name=f"Z{sb}", tag="Z")
                nc.scalar.activation(E[:], L[:], Act.Exp,
                                     bias=negmx[:], scale=1.0, accum_out=Z[:])
                rz = small.tile([P, 1], F32, name=f"rz{sb}", tag="rz")
                nc.vector.reciprocal(rz[:], Z[:])
                Et = etpool.tile([P, SD], F16, name=f"Et{sb}", tag="Et")
                nc.scalar.activation(Et[:], E[:], Act.Copy, scale=rz[:])
                nc.vector.tensor_add(cp[:], cp[:], Et[:])

        # ---- tail: w -> u -> C ----
        with tc.tile_pool(name="tps", bufs=2, space="PSUM") as tps, \
             tc.tile_pool(name="ups", bufs=2, space="PSUM") as ups, \
             tc.tile_pool(name="dps", bufs=2, space="PSUM") as dps, \
             tc.tile_pool(name="tmisc", bufs=1) as tmisc:
            # Dout = xsum @ W_vc / (SD*32)  (fp32; fills PE idle in the tail)
            for ch in range(2):
                pd = dps.tile([1, CH], F32, name=f"pd_{ch}", tag="pd")
                for f in range(DB):
                    nc.tensor.matmul(pd[:], xsum[:, f:f + 1],
                                     wvc32[f][:, ch * CH:(ch + 1) * CH],
                                     start=(f == 0), stop=(f == DB - 1))
                nc.scalar.activation(out_sb[0:1, D + ch * CH:D + (ch + 1) * CH], pd[:],
                                     Act.Copy, scale=1.0 / (SD * 32.0))
            wT = tmisc.tile([P, TB], F32, name="wT")
            for k in range(TB):
                tp = tps.tile([P, P], F16, name=f"tp{k}", tag="tp")
                nc.tensor.transpose(tp[:], cp[:, k * P:(k + 1) * P], ident[:])
                nc.vector.tensor_reduce(wT[:, k:k + 1], tp[:], AxX, Alu.add)
            nc.vector.tensor_copy(wT16[:], wT[:])

            # u = w @ x_d
            for ch in range(2):
                pu = ups.tile([1, CH], F32, name=f"pu{ch}", tag="pu")
                for t in range(TB):
                    nc.tensor.matmul(pu[:], wT16[:, t:t + 1],
                                     xdn[t][:, ch * CH:(ch + 1) * CH],
                                     start=(t == 0), stop=(t == TB - 1))
                if ch == 0:
                    nc.scalar.activation(u32[0:1, ch * CH:(ch + 1) * CH], pu[:], Act.Copy)
                else:
                    nc.vector.tensor_copy(u32[0:1, ch * CH:(ch + 1) * CH], pu[:])

            # transpose u row -> uT16 columns
            put = tps.tile([P, DB], F32, name="put", tag="put")
            for e in range(DB):
                nc.tensor.transpose(put[:, e:e + 1], u32[0:1, e * P:(e + 1) * P],
                                    ones32[:])
            nc.scalar.activation(uT16[:], put[:], Act.Copy)

            # C = u @ W_vd / (SC*32)
            for ch in range(2):
                pv = ups.tile([1, CH], F32, name=f"pv{ch}", tag="pu")
                for f in range(DB):
                    nc.tensor.matmul(pv[:], uT16[:, f:f + 1],
                                     wvd16[f][:, ch * CH:(ch + 1) * CH],
                                     start=(f == 0), stop=(f == DB - 1))
                if ch == 0:
                    nc.scalar.activation(out_sb[0:1, ch * CH:(ch + 1) * CH], pv[:],
                                         Act.Copy, scale=1.0 / (SC * 32.0))
                else:
                    nc.vector.tensor_scalar_mul(out_sb[0:1, ch * CH:(ch + 1) * CH],
                                                pv[:], 1.0 / (SC * 32.0))
            nc.sync.dma_start(out_d, out_sb[:])

    _split_excess_waits(nc, mybir)
    return nc


def kernel(x_c, x_d, W_qc, W_vc, W_kd, W_vd):
    from concourse.bass_utils import run_bass_kernel_spmd
    nc = _build()
    in_maps = []
    for b in range(B):
        in_maps.append({
            "x_c": np.ascontiguousarray(x_c[b]),
            "x_d": np.ascontiguousarray(x_d[b]),
            "W_qc": np.asarray(W_qc), "W_vc": np.asarray(W_vc),
            "W_kd": np.asarray(W_kd), "W_vd": np.asarray(W_vd),
        })
    res = run_bass_kernel_spmd(nc, in_maps, list(range(B))).results
    C = np.empty((B, D), dtype=np.float32)
    Dout = np.empty((B, D), dtype=np.float32)
    for b in range(B):
        o = res[b]["out"]
        C[b] = o[0, :D]
        Dout[b] = o[0, D:]
    return (C, Dout)
